# revision 14
# baseline (speedup 1.0000x reference)
"""Trainium2 Bass kernel for nn_CLGNN_Model (3-layer GCN + MLP head + log_softmax).

Sharding: nodes are partitioned across 8 NeuronCores (12500 each).  Per GCN
layer, each core computes z = h @ W for its own nodes, scales rows by
dinv = rsqrt(deg), casts to fp8e4 and AllGathers the resulting "message table"
[100352, 256].  The table is laid out in 4 window-quarters so the AllGather
splits into 4 chunk-wise collectives that overlap with compute: phase M of
layer L runs interleaved inside phase A of layer L-1 (driven by postlude
completion), and each quarter's AllGather is issued as soon as its windows
are produced.  Edges are assigned to the core owning their destination;
the aggregation  acc[dst] = sum_{e->dst} g[src_e]  is computed with
dma_gather (int16-indexed row gather from the table; each quarter is an
int16-addressable chunk) followed by 0/1-indicator matmuls on the
TensorEngine (fp8 DoubleRow pairs two 128-row groups per matmul) into a
PSUM accumulator per 128-destination window.  Indicator matrices are
precomputed on the host and DMAed in.  Self-loops (weight 2.0) are folded
into the postlude on the Vector engine.  The MLP head + log_softmax run
interleaved inside phase A of the last layer.  The instruction stream is
identical across cores (group counts are maxed over cores, short cells
padded with sentinel edges) so one SPMD program serves all 8 cores; only
the data arrays differ.
"""
import sys
import os
import hashlib
from dataclasses import dataclass

sys.path.insert(0, "/opt/trn_rl_repo")

import numpy as np
import ml_dtypes

BF16 = ml_dtypes.bfloat16
F8 = ml_dtypes.float8_e4m3

# ----------------------------------------------------------------------------
# configuration
# ----------------------------------------------------------------------------


@dataclass(frozen=True)
class Cfg:
    N: int = 100000           # total nodes
    NFEAT: int = 512
    NLABEL: int = 64
    NHID: int = 256
    NCORES: int = 8
    P: int = 128
    SW: int = 3               # windows per superwindow

    @property
    def NOWN(self):           # nodes per core
        return self.N // self.NCORES

    @property
    def NW(self):             # 128-windows per core
        return (self.NOWN + self.P - 1) // self.P

    @property
    def NOWN_PAD(self):
        return self.NW * self.P

    @property
    def TBL_ROWS(self):
        return self.NCORES * self.NOWN_PAD

    @property
    def NCHUNK(self):
        return 4

    @property
    def QW(self):             # window-quarter boundaries
        base = self.NW // self.NCHUNK
        rem = self.NW - base * self.NCHUNK
        qs = [base + (1 if i < rem else 0) for i in range(self.NCHUNK)]
        return [0] + list(np.cumsum(qs))

    @property
    def QROW(self):           # table row offsets per quarter
        return [self.NCORES * self.P * q for q in self.QW]

    @property
    def NSW(self):
        return (self.NW + self.SW - 1) // self.SW

    @property
    def DIN(self):            # GCN layer-0 input dim
        return self.NFEAT + self.NLABEL

    @property
    def KIN0(self):           # 128-chunks of DIN (padded)
        return (self.DIN + self.P - 1) // self.P


FULL = Cfg()
PAD_DR = 200                 # dst_rel sentinel for padded edge slots

# ----------------------------------------------------------------------------
# host-side preprocessing
# ----------------------------------------------------------------------------


def _build_feats(cfg, x, y, idx_labeled):
    n = x.shape[0]
    idx = np.full((n,), cfg.NLABEL + 2, np.int64)
    idx[idx_labeled] = y[idx_labeled]
    feats = np.zeros((n, cfg.NLABEL), np.float32)
    lab = idx < cfg.NLABEL
    feats[np.nonzero(lab)[0], idx[lab]] = 1.0
    return np.concatenate([x, feats], axis=1)


def _trow(cfg, nodes):
    """Global node id -> table row under the window-quarter layout."""
    QW = np.array(cfg.QW)
    QROW = np.array(cfg.QROW)
    k = nodes // cfg.NOWN
    local = nodes % cfg.NOWN
    w = local // cfg.P
    p = local % cfg.P
    q = np.searchsorted(QW, w, side="right") - 1
    nwq = (QW[q + 1] - QW[q])
    return QROW[q] + k * nwq * cfg.P + (w - QW[q]) * cfg.P + p, q


def _build_schedule(cfg, adj):
    """Device-independent schedule + per-device index/indicator arrays."""
    P = cfg.P
    src = adj[0].astype(np.int64)
    dst = adj[1].astype(np.int64)

    indeg = np.bincount(dst, minlength=cfg.N).astype(np.float32)
    deg_full = indeg + 2.0

    # cells in schedule order: sw asc -> chunk asc -> window asc
    ncells = sum(cfg.NCHUNK * (min((s + 1) * cfg.SW, cfg.NW) - s * cfg.SW)
                 for s in range(cfg.NSW))

    dev_edges = []        # (cell, src_local, dst_rel) arrays per device
    counts = np.zeros((cfg.NCORES, ncells), np.int64)
    sw_sizes = [min((s + 1) * cfg.SW, cfg.NW) - s * cfg.SW
                for s in range(cfg.NSW)]
    sw_off_arr = np.cumsum([0] + [sz * cfg.NCHUNK for sz in sw_sizes])[:-1]
    sw_sz_arr = np.array(sw_sizes)
    QROW = np.array(cfg.QROW)
    for d in range(cfg.NCORES):
        mask = (dst // cfg.NOWN) == d
        es = src[mask]
        ed = dst[mask]
        dl = ed - d * cfg.NOWN
        w = dl // P
        dst_rel = (dl % P).astype(np.uint8)
        trow, c = _trow(cfg, es)
        src_local = (trow - QROW[c]).astype(np.int64)
        sw_of = w // cfg.SW
        w_in_sw = w - sw_of * cfg.SW
        cidv = sw_off_arr[sw_of] + c * sw_sz_arr[sw_of] + w_in_sw
        np.add.at(counts[d], cidv, 1)
        dev_edges.append((cidv, src_local, dst_rel))

    G = (counts.max(axis=0) + P - 1) // P   # groups per cell (0 if empty)
    cap = G * P
    cell_off = np.concatenate([[0], np.cumsum(cap)])       # edge offsets
    G_off = np.concatenate([[0], np.cumsum(G)])            # group offsets
    G_total = int(G.sum())
    total = int(cap.sum())

    # ---- per-sw gather calls + pair stream -------------------------------
    sw_calls = []         # per sw: [(chunk, ng, goff_rel)] (calls split in 2)
    sw_g_off = []         # sw group base (global)
    sw_g_cnt = []
    sw_pairs = []         # per sw: [(j_rel, w, start, stop)]
    sw_pair_off = []      # pair base (global, into indicator tensor)
    pair_groups = []      # global: [(ga, gb)] absolute group id or -1
    ci = 0
    for s in range(cfg.NSW):
        ws = list(range(s * cfg.SW, min((s + 1) * cfg.SW, cfg.NW)))
        g0 = G_off[ci]
        calls = []
        ci_start = ci
        for c in range(cfg.NCHUNK):
            ng = 0
            goff = G_off[ci]
            for _ in ws:
                ng += int(G[ci])
                ci += 1
            goff = int(goff - g0)
            nsplit = 4
            done = 0
            for si in range(nsplit):
                part = (ng - done + nsplit - si - 1) // (nsplit - si)
                if part:
                    calls.append((c, part, goff + done))
                done += part
        gcnt = int(G_off[ci] - g0)
        assert gcnt >= 2, f"superwindow {s} has <2 groups"
        # pair stream in cell order
        stream = []       # (j_rel, w, ga, gb)
        cj = ci_start
        for c in range(cfg.NCHUNK):
            for w in ws:
                off = int(G_off[cj] - g0)
                Gc = int(G[cj])
                for p in range(Gc // 2):
                    stream.append((off + 2 * p, w,
                                   int(G_off[cj]) + 2 * p,
                                   int(G_off[cj]) + 2 * p + 1))
                if Gc % 2:
                    gl = int(G_off[cj]) + Gc - 1
                    if Gc >= 3:
                        stream.append((off + Gc - 2, w, -1, gl))
                    elif off >= 1:
                        stream.append((off - 1, w, -1, gl))
                    else:
                        stream.append((0, w, gl, -1))
                cj += 1
        # start/stop flags per window
        first = {}
        last = {}
        for i, (j, w, ga, gb) in enumerate(stream):
            if w not in first:
                first[w] = i
            last[w] = i
        sw_pairs.append([(j, w, int(i == first[w]), int(i == last[w]))
                         for i, (j, w, ga, gb) in enumerate(stream)])
        sw_pair_off.append(len(pair_groups))
        pair_groups.extend((ga, gb) for (j, w, ga, gb) in stream)
        sw_calls.append(calls)
        sw_g_off.append(int(g0))
        sw_g_cnt.append(gcnt)
    M_total = len(pair_groups)
    PAIRS_MAX = max(len(p) for p in sw_pairs)
    G_SW_MAX = max(sw_g_cnt)
    pair_groups = np.array(pair_groups, np.int64)          # [M, 2]

    # per-device data arrays
    dev_idx = []
    dev_ind = []
    dev_deg = []
    onehot = np.zeros((256, P), F8)
    onehot[np.arange(P), np.arange(P)] = 1.0
    for d in range(cfg.NCORES):
        cidv, src_local, dst_rel = dev_edges[d]
        # ascending src within each cell -> better HBM locality for gathers
        order = np.lexsort((src_local, cidv))
        cid_s = cidv[order]
        starts = np.searchsorted(cid_s, np.arange(ncells))
        within = np.arange(len(cid_s)) - starts[cid_s]
        pos = cell_off[cid_s] + within
        idx_flat = np.zeros(total, np.int64)
        dr_flat = np.full(total, PAD_DR, np.uint8)
        idx_flat[pos] = src_local[order]
        dr_flat[pos] = dst_rel[order]
        # wrapped int16 layout [128, total//16]
        a = idx_flat.reshape(total // 16, 16).T.astype(np.int16)
        dev_idx.append(np.ascontiguousarray(np.tile(a, (8, 1))))
        # indicators: [P(edge), M, 2, P(dst)] fp8
        dr_groups = dr_flat.reshape(G_total, P)
        slot_dr = np.full((M_total, 2, P), PAD_DR, np.uint8)
        for k in range(2):
            gk = pair_groups[:, k]
            valid = gk >= 0
            slot_dr[valid, k, :] = dr_groups[gk[valid]]
        ind = onehot[slot_dr]                   # [M, 2, Pedge, Pdst]
        dev_ind.append(np.ascontiguousarray(ind.transpose(2, 0, 1, 3)))
        dg = np.full((cfg.NOWN_PAD,), 1.0, np.float32)
        dg[:cfg.NOWN] = deg_full[d * cfg.NOWN:(d + 1) * cfg.NOWN]
        dev_deg.append(np.ascontiguousarray(
            dg.reshape(cfg.NW, P).T))          # [128, NW]

    sched = dict(
        sw_calls=sw_calls, sw_g_off=sw_g_off, sw_g_cnt=sw_g_cnt,
        sw_pairs=sw_pairs, sw_pair_off=sw_pair_off,
        G_SW_MAX=int(G_SW_MAX), PAIRS_MAX=int(PAIRS_MAX),
        G_total=G_total, M_total=M_total, S_total=G_total * 8,
    )
    return sched, dev_idx, dev_ind, dev_deg


def _pack_h0(cfg, h0_dev):
    """[NOWN_PAD, DIN] f32 -> [NW, 128, KIN0*128] bf16 lhsT-packed."""
    dpad = cfg.KIN0 * cfg.P
    h = np.zeros((cfg.NOWN_PAD, dpad), np.float32)
    h[:, :cfg.DIN] = h0_dev
    v = h.reshape(cfg.NW, cfg.P, cfg.KIN0, cfg.P)      # t, nc, kc, p
    return np.ascontiguousarray(v.transpose(0, 3, 2, 1)
                                .reshape(cfg.NW, cfg.P, cfg.KIN0 * cfg.P)
                                .astype(BF16))


def _pack_w(W, kin_chunks, p=128):
    """[K, O] -> [kin_chunks, 128, O] bf16 (zero-padded)."""
    K, O = W.shape
    Wp = np.zeros((kin_chunks * p, O), np.float32)
    Wp[:K] = W
    return np.ascontiguousarray(
        Wp.reshape(kin_chunks, p, O).astype(BF16))


def _bcast(b, p=128):
    return np.ascontiguousarray(np.broadcast_to(
        b.astype(np.float32)[None, :], (p, len(b))).copy())


# ----------------------------------------------------------------------------
# Bass program
# ----------------------------------------------------------------------------


def _build_nc(cfg, sched):
    from concourse import bass, mybir, tile, bacc
    from concourse.masks import make_identity
    from contextlib import ExitStack

    fp32 = mybir.dt.float32
    bf16 = mybir.dt.bfloat16
    fp8 = mybir.dt.float8e4
    i16 = mybir.dt.int16
    DRmode = mybir.MatmulPerfMode.DoubleRow
    P = cfg.P
    NH = cfg.NHID
    NW = cfg.NW
    SW = cfg.SW
    KIN0 = cfg.KIN0
    QW = cfg.QW
    QROW = cfg.QROW
    G_SW_MAX = sched["G_SW_MAX"]
    PAIRS_MAX = sched["PAIRS_MAX"]
    # AllGather quarter q is issued after this phase-M window index
    ag_after_w = {QW[q + 1] - 1: q for q in range(cfg.NCHUNK)}

    nc = bacc.Bacc("TRN2", debug=False, num_swdge_queues=4)

    hT0_d = nc.dram_tensor("hT0", [NW, P, KIN0 * P], bf16, kind="ExternalInput")
    idx_d = nc.dram_tensor("idx", [P, sched["S_total"]], i16, kind="ExternalInput")
    ind_d = nc.dram_tensor("ind", [P, sched["M_total"], 2, P], fp8,
                           kind="ExternalInput")
    deg_d = nc.dram_tensor("deg", [P, NW], fp32, kind="ExternalInput")
    w0_d = nc.dram_tensor("w0", [KIN0, P, NH], bf16, kind="ExternalInput")
    w12_d = nc.dram_tensor("w12", [2, 2, P, NH], bf16, kind="ExternalInput")
    wm0_d = nc.dram_tensor("wm0", [2, P, 2 * NH], bf16, kind="ExternalInput")
    wm1_d = nc.dram_tensor("wm1", [4, P, 64], bf16, kind="ExternalInput")
    b012_d = nc.dram_tensor("b012", [3, P, NH], fp32, kind="ExternalInput")
    bm0_d = nc.dram_tensor("bm0", [P, 2 * NH], fp32, kind="ExternalInput")
    bm1_d = nc.dram_tensor("bm1", [P, 64], fp32, kind="ExternalInput")
    out_d = nc.dram_tensor("out", [NW, P, 64], fp32, kind="ExternalOutput")

    with tile.TileContext(nc) as tc, ExitStack() as ctx:
        const = ctx.enter_context(tc.tile_pool(name="const", bufs=1))
        ht = ctx.enter_context(tc.tile_pool(name="ht", bufs=1))
        work = ctx.enter_context(tc.tile_pool(name="work", bufs=2))
        tri = ctx.enter_context(tc.tile_pool(name="tri", bufs=2))
        pacc = ctx.enter_context(tc.tile_pool(name="pacc", bufs=6, space="PSUM"))
        pmz = ctx.enter_context(tc.tile_pool(name="pmz", bufs=2, space="PSUM"))
        dram = ctx.enter_context(tc.tile_pool(name="dram", bufs=1, space="DRAM"))

        # ---- constants -----------------------------------------------------
        ident = const.tile([P, P], bf16, tag="ident")
        make_identity(nc, ident[:])
        deg_sb = const.tile([P, NW], fp32, tag="deg")
        nc.sync.dma_start(deg_sb[:], deg_d[:])
        dinv = const.tile([P, NW], fp32, tag="dinv")
        nc.scalar.sqrt(deg_sb[:], deg_sb[:])
        nc.vector.reciprocal(dinv[:], deg_sb[:])
        dinv2 = const.tile([P, NW], fp32, tag="dinv2")
        nc.vector.tensor_scalar_mul(dinv2[:], dinv[:], 2.0)

        w0_sb = const.tile([P, KIN0, NH], bf16, tag="w0")
        nc.sync.dma_start(w0_sb[:], w0_d[:].rearrange("k p o -> p k o"))
        w12_sb = const.tile([P, 2, 2, NH], bf16, tag="w12")
        nc.sync.dma_start(w12_sb[:], w12_d[:].rearrange("l k p o -> p l k o"))
        wm0_sb = const.tile([P, 2, 2 * NH], bf16, tag="wm0")
        nc.sync.dma_start(wm0_sb[:], wm0_d[:].rearrange("k p o -> p k o"))
        wm1_sb = const.tile([P, 4, 64], bf16, tag="wm1")
        nc.sync.dma_start(wm1_sb[:], wm1_d[:].rearrange("k p o -> p k o"))
        b012_sb = const.tile([P, 3, NH], fp32, tag="b012")
        nc.sync.dma_start(b012_sb[:], b012_d[:].rearrange("l p o -> p l o"))
        bm0_sb = const.tile([P, 2 * NH], fp32, tag="bm0")
        nc.sync.dma_start(bm0_sb[:], bm0_d[:])
        bm1_sb = const.tile([P, 64], fp32, tag="bm1")
        nc.sync.dma_start(bm1_sb[:], bm1_d[:])

        # persistent transposed activations, 2 feature chunks of 128
        hTa = ht.tile([P, NW * P], bf16, tag="hTa")
        hTb = ht.tile([P, NW * P], bf16, tag="hTb")

        ag_ins = [dram.tile([NW, P, NH], fp8, tag=f"agin{l}",
                            name=f"agin{l}") for l in range(3)]
        tables = [[dram.tile([QROW[q + 1] - QROW[q], NH], fp8,
                             tag=f"tbl{l}q{q}", name=f"tbl{l}q{q}",
                             addr_space="Shared")
                   for q in range(cfg.NCHUNK)] for l in range(3)]

        def issue_ag(layer, q):
            nc.gpsimd.collective_compute(
                "AllGather", mybir.AluOpType.bypass,
                ins=[ag_ins[layer][QW[q]:QW[q + 1]].opt()],
                outs=[tables[layer][q][:].opt()],
                replica_groups=[list(range(cfg.NCORES))],
            )

        def phase_m_window(layer, t, stage, si):
            """z = h@W for window t -> stage[:, si, :] (fp8, dinv-scaled)."""
            nkin = 2
            psum_z = pacc.tile([P, NH], fp32, tag="acc", name="psum_z")
            for kc in range(nkin):
                lhsT = (hTa if kc == 0 else hTb)[:, t * P:(t + 1) * P]
                rhs = w12_sb[:, layer - 1, kc, :]
                nc.tensor.matmul(psum_z[:], lhsT, rhs,
                                 start=(kc == 0), stop=(kc == nkin - 1))
            nc.vector.tensor_scalar_mul(
                stage[:, si, :], psum_z[:], dinv[:, t:t + 1])

        def head_windows(w_lo, w_hi):
            """MLP head + log_softmax for windows [w_lo, w_hi)."""
            nwv = w_hi - w_lo
            mbs = []
            for j in range(nwv):
                t = w_lo + j
                psum_m = pacc.tile([P, 2 * NH], fp32, tag="acc",
                                   name="psum_m")
                for kc in range(2):
                    lhsT = (hTa if kc == 0 else hTb)[:, t * P:(t + 1) * P]
                    nc.tensor.matmul(psum_m[:], lhsT, wm0_sb[:, kc, :],
                                     start=(kc == 0), stop=(kc == 1))
                z0 = tri.tile([P, 2 * NH], fp32, tag="z0")
                nc.vector.tensor_add(z0[:], psum_m[:], bm0_sb[:])
                # elu(z) = relu(z) + min(exp(z) - 1, 0)
                ex = tri.tile([P, 2 * NH], bf16, tag="ex")
                nc.scalar.activation(ex[:], z0[:],
                                     mybir.ActivationFunctionType.Exp)
                nc.vector.tensor_scalar(
                    out=ex[:], in0=ex[:], scalar1=1.0, scalar2=0.0,
                    op0=mybir.AluOpType.subtract, op1=mybir.AluOpType.min)
                mb = tri.tile([P, 2 * NH], bf16, tag="mb", name="mb",
                              bufs=SW + 2)
                nc.scalar.activation(mb[:], z0[:],
                                     mybir.ActivationFunctionType.Relu)
                nc.vector.tensor_add(mb[:], mb[:], ex[:])
                mbs.append(mb)
            lg8 = tri.tile([P, SW, 64], fp32, tag="lg8")
            for j in range(nwv):
                mT = tri.tile([P, 4, P], bf16, tag="mT", name="mT")
                for q in range(4):
                    ptp = pmz.tile([P, P], bf16, tag="mz", name="ptp")
                    nc.tensor.transpose(ptp[:], mbs[j][:, q * P:(q + 1) * P],
                                        ident[:])
                    nc.scalar.activation(mT[:, q, :], ptp[:],
                                         mybir.ActivationFunctionType.Copy)
                psum_l = pacc.tile([P, 64], fp32, tag="acc", name="psum_l")
                for q in range(4):
                    nc.tensor.matmul(psum_l[:], mT[:, q, :], wm1_sb[:, q, :],
                                     start=(q == 0), stop=(q == 3))
                nc.vector.tensor_add(lg8[:, j, :], psum_l[:], bm1_sb[:])
            # batched log_softmax over the wave
            mx8 = tri.tile([P, SW, 1], fp32, tag="mx8")
            nc.vector.tensor_reduce(mx8[:, :nwv, :], lg8[:, :nwv, :],
                                    axis=mybir.AxisListType.X,
                                    op=mybir.AluOpType.max)
            nc.vector.tensor_tensor(
                out=lg8[:, :nwv, :], in0=lg8[:, :nwv, :],
                in1=mx8[:, :nwv, :].to_broadcast([P, nwv, 64]),
                op=mybir.AluOpType.subtract)
            ex8 = tri.tile([P, SW, 64], bf16, tag="ex8")
            nc.scalar.activation(ex8[:, :nwv, :], lg8[:, :nwv, :],
                                 mybir.ActivationFunctionType.Exp)
            se8 = tri.tile([P, SW, 1], fp32, tag="se8")
            nc.vector.tensor_reduce(se8[:, :nwv, :], ex8[:, :nwv, :],
                                    axis=mybir.AxisListType.X,
                                    op=mybir.AluOpType.add)
            ln8 = tri.tile([P, SW, 1], fp32, tag="ln8")
            nc.scalar.activation(ln8[:, :nwv, :], se8[:, :nwv, :],
                                 mybir.ActivationFunctionType.Ln)
            out_stage = tri.tile([P, SW, 64], fp32, tag="ostage")
            nc.vector.tensor_tensor(
                out=out_stage[:, :nwv, :], in0=lg8[:, :nwv, :],
                in1=ln8[:, :nwv, :].to_broadcast([P, nwv, 64]),
                op=mybir.AluOpType.subtract)
            nc.sync.dma_start(
                out_d[w_lo:w_hi].rearrange("t p f -> p t f"),
                out_stage[:, :nwv, :])

        # ---- layer-0 phase M (standalone, AG quarters issued inline) ------
        SLAB = 4
        g_stage = None
        stage_base = 0
        h0slab = None
        for t in range(NW):
            if t % SLAB == 0:
                nsl = min(SLAB, NW - t)
                h0slab = tri.tile([P, SLAB, KIN0 * P], bf16,
                                  tag="h0slab", bufs=2)
                nc.sync.dma_start(
                    h0slab[:, :nsl, :],
                    hT0_d[t:t + nsl].rearrange("t p f -> p t f"))
            if t == stage_base:
                g_stage = tri.tile([P, 8, NH], fp8, tag="stage")
            psum_z = pacc.tile([P, NH], fp32, tag="acc", name="psum_z")
            for kc in range(KIN0):
                nc.tensor.matmul(psum_z[:],
                                 h0slab[:, t % SLAB, kc * P:(kc + 1) * P],
                                 w0_sb[:, kc, :],
                                 start=(kc == 0), stop=(kc == KIN0 - 1))
            nc.vector.tensor_scalar_mul(
                g_stage[:, t - stage_base, :], psum_z[:], dinv[:, t:t + 1])
            flush = (t - stage_base == 7) or (t == NW - 1) or (t in ag_after_w)
            if flush:
                nb = t - stage_base + 1
                nc.sync.dma_start(
                    ag_ins[0][stage_base:stage_base + nb]
                    .rearrange("t p f -> p t f"),
                    g_stage[:, :nb, :])
                stage_base = t + 1
            if t in ag_after_w:
                issue_ag(0, ag_after_w[t])

        # ---- 3 GCN layers: phase A (+ next phase M / head interleaved) ----
        for layer in range(3):
            table = tables[layer]
            ag_in = ag_ins[layer]

            def postlude(w, acc, gown, w_lo):
                # h = relu(acc*dinv + gown*2*dinv + bias); -> hTa/hTb
                tmp = tri.tile([P, NH], fp32, tag="pl_tmp", name="pl_tmp")
                if acc is not None:
                    nc.vector.scalar_tensor_tensor(
                        out=tmp[:], in0=acc[:],
                        scalar=dinv[:, w:w + 1],
                        in1=b012_sb[:, layer, :],
                        op0=mybir.AluOpType.mult,
                        op1=mybir.AluOpType.add)
                    nc.vector.scalar_tensor_tensor(
                        out=tmp[:], in0=gown[:, w - w_lo, :],
                        scalar=dinv2[:, w:w + 1],
                        in1=tmp[:],
                        op0=mybir.AluOpType.mult,
                        op1=mybir.AluOpType.add)
                else:
                    nc.vector.scalar_tensor_tensor(
                        out=tmp[:], in0=gown[:, w - w_lo, :],
                        scalar=dinv2[:, w:w + 1],
                        in1=b012_sb[:, layer, :],
                        op0=mybir.AluOpType.mult,
                        op1=mybir.AluOpType.add)
                hbf = tri.tile([P, NH], bf16, tag="pl_hbf", name="pl_hbf")
                nc.scalar.activation(
                    hbf[:], tmp[:], mybir.ActivationFunctionType.Relu)
                for half, dst_t in ((0, hTa), (1, hTb)):
                    ptp = pmz.tile([P, P], bf16, tag="mz", name="ptp")
                    nc.tensor.transpose(
                        ptp[:], hbf[:, half * P:(half + 1) * P], ident[:])
                    nc.scalar.activation(
                        dst_t[:, w * P:(w + 1) * P], ptp[:],
                        mybir.ActivationFunctionType.Copy)

            for s in range(cfg.NSW):
                w_lo = s * SW
                w_hi = min((s + 1) * SW, NW)
                nwin = w_hi - w_lo
                g0 = sched["sw_g_off"][s]
                gcnt = sched["sw_g_cnt"][s]
                pairs = sched["sw_pairs"][s]
                p0 = sched["sw_pair_off"][s]

                idx_sb = work.tile([P, G_SW_MAX * 8], i16, tag="idx")
                nc.sync.dma_start(idx_sb[:, :gcnt * 8],
                                  idx_d[:, g0 * 8:(g0 + gcnt) * 8])
                ind_sb = work.tile([P, PAIRS_MAX, 2, P], fp8, tag="ind")
                nc.sync.dma_start(ind_sb[:, :len(pairs), :, :],
                                  ind_d[:, p0:p0 + len(pairs), :, :])
                gown = work.tile([P, SW, NH], fp8, tag="gown", bufs=3)
                nc.sync.dma_start(
                    gown[:, :nwin, :],
                    ag_in[w_lo:w_hi].rearrange("t p f -> p t f"))
                gath = work.tile([P, G_SW_MAX + 1, NH], fp8, tag="gath",
                                 bufs=5)
                for ci_call, (c, ng, goff) in enumerate(
                        sched["sw_calls"][s]):
                    nc.gpsimd.dma_gather(
                        out_ap=gath[:, goff:goff + ng, :],
                        in_ap=table[c][:],
                        idxs_ap=idx_sb[:, goff * 8:(goff + ng) * 8],
                        num_idxs=ng * P,
                        num_idxs_reg=ng * P,
                        elem_size=NH,
                        single_packet=True,
                        queue_num=ci_call % 4,
                    )
                accs = {}
                for pl, (j, w, st, sp) in enumerate(pairs):
                    if st:
                        accs[w] = pacc.tile([P, NH], fp32, tag="acc",
                                            name="acc")
                    nc.tensor.matmul(
                        accs[w][:], ind_sb[:, pl, :, :],
                        gath[:, j:j + 2, :],
                        start=bool(st), stop=bool(sp), perf_mode=DRmode)
                    if sp:
                        postlude(w, accs[w], gown, w_lo)
                for w in range(w_lo, w_hi):
                    if w not in accs:
                        postlude(w, None, gown, w_lo)

                # interleaved next-layer phase M / final head
                if layer < 2:
                    stage = tri.tile([P, SW, NH], fp8, tag="stage2")
                    for w in range(w_lo, w_hi):
                        phase_m_window(layer + 1, w, stage, w - w_lo)
                    nc.sync.dma_start(
                        ag_ins[layer + 1][w_lo:w_hi]
                        .rearrange("t p f -> p t f"),
                        stage[:, :nwin, :])
                    for w in range(w_lo, w_hi):
                        if w in ag_after_w:
                            issue_ag(layer + 1, ag_after_w[w])
                else:
                    head_windows(w_lo, w_hi)

    nc.compile()
    return nc


# ----------------------------------------------------------------------------
# entry point
# ----------------------------------------------------------------------------

_NC_CACHE = {}
TRACE = False
TRACE_KW = {}
LAST_RESULT = None


def _prepare(cfg, inputs):
    x = np.asarray(inputs["x"], np.float32)
    y = np.asarray(inputs["y"])
    adj = np.asarray(inputs["adj"])
    idx_labeled = np.asarray(inputs["idx_labeled"])

    h0 = _build_feats(cfg, x, y, idx_labeled)
    sched, dev_idx, dev_ind, dev_deg = _build_schedule(cfg, adj)

    W0 = _pack_w(np.asarray(inputs["W0"], np.float32), cfg.KIN0)
    W1 = _pack_w(np.asarray(inputs["W1"], np.float32), 2)
    W2 = _pack_w(np.asarray(inputs["W2"], np.float32), 2)
    w12 = np.ascontiguousarray(np.stack([W1, W2]))
    Wm0 = _pack_w(np.asarray(inputs["Wm0"], np.float32), 2)
    Wm1 = _pack_w(np.asarray(inputs["Wm1"], np.float32), 4)
    b012 = np.ascontiguousarray(np.stack(
        [_bcast(np.asarray(inputs[k], np.float32)) for k in ("b0", "b1", "b2")]))
    bm0 = _bcast(np.asarray(inputs["bm0"], np.float32))
    bm1 = _bcast(np.asarray(inputs["bm1"], np.float32))

    in_maps = []
    for d in range(cfg.NCORES):
        h0_dev = np.zeros((cfg.NOWN_PAD, cfg.DIN), np.float32)
        h0_dev[:cfg.NOWN] = h0[d * cfg.NOWN:(d + 1) * cfg.NOWN]
        in_maps.append(dict(
            hT0=_pack_h0(cfg, h0_dev),
            idx=dev_idx[d], ind=dev_ind[d], deg=dev_deg[d],
            w0=W0, w12=w12, wm0=Wm0, wm1=Wm1,
            b012=b012, bm0=bm0, bm1=bm1,
        ))
    return sched, in_maps


def run(cfg, inputs):
    global LAST_RESULT
    from concourse.bass_utils import run_bass_kernel_spmd

    sched, in_maps = _prepare(cfg, inputs)
    key = (cfg, hashlib.sha1(
        np.asarray(inputs["adj"]).tobytes()).hexdigest())
    if key not in _NC_CACHE:
        _NC_CACHE[key] = _build_nc(cfg, sched)
    nc = _NC_CACHE[key]

    res = run_bass_kernel_spmd(
        nc, in_maps, core_ids=list(range(cfg.NCORES)),
        trace=TRACE, **TRACE_KW)
    LAST_RESULT = res
    outs = []
    for d in range(cfg.NCORES):
        o = res.results[d]["out"].reshape(cfg.NOWN_PAD, 64)
        outs.append(o[:cfg.NOWN])
    return np.ascontiguousarray(np.concatenate(outs, axis=0))


def kernel(**inputs) -> np.ndarray:
    return run(FULL, inputs)


# revision 16
# speedup vs baseline: 1.1126x; 1.1126x over previous
"""Trainium2 Bass kernel for nn_CLGNN_Model (3-layer GCN + MLP head + log_softmax).

Sharding: nodes are partitioned across 8 NeuronCores (12500 each).  Per GCN
layer, each core computes z = h @ W for its own nodes, scales rows by
dinv = rsqrt(deg), casts to fp8e4 and AllGathers the resulting "message table"
[100352, 256].  The table is laid out in 4 window-quarters so the AllGather
splits into 4 chunk-wise collectives that overlap with compute: phase M of
layer L runs interleaved inside phase A of layer L-1 (driven by postlude
completion), and each quarter's AllGather is issued as soon as its windows
are produced.  Edges are assigned to the core owning their destination;
the aggregation  acc[dst] = sum_{e->dst} g[src_e]  is computed with
dma_gather (int16-indexed row gather from the table; each quarter is an
int16-addressable chunk) followed by 0/1-indicator matmuls on the
TensorEngine (fp8 DoubleRow pairs two 128-row groups per matmul) into a
PSUM accumulator per 128-destination window.  Indicator matrices are
precomputed on the host and DMAed in.  Self-loops (weight 2.0) are folded
into the postlude on the Vector engine.  The MLP head + log_softmax run
interleaved inside phase A of the last layer.  The instruction stream is
identical across cores (group counts are maxed over cores, short cells
padded with sentinel edges) so one SPMD program serves all 8 cores; only
the data arrays differ.
"""
import sys
import os
import hashlib
from dataclasses import dataclass

sys.path.insert(0, "/opt/trn_rl_repo")

import numpy as np
import ml_dtypes

BF16 = ml_dtypes.bfloat16
F8 = ml_dtypes.float8_e4m3

# ----------------------------------------------------------------------------
# configuration
# ----------------------------------------------------------------------------


@dataclass(frozen=True)
class Cfg:
    N: int = 100000           # total nodes
    NFEAT: int = 512
    NLABEL: int = 64
    NHID: int = 256
    NCORES: int = 8
    P: int = 128
    SW: int = 3               # windows per superwindow

    @property
    def NOWN(self):           # nodes per core
        return self.N // self.NCORES

    @property
    def NW(self):             # 128-windows per core
        return (self.NOWN + self.P - 1) // self.P

    @property
    def NOWN_PAD(self):
        return self.NW * self.P

    @property
    def TBL_ROWS(self):
        return self.NCORES * self.NOWN_PAD

    @property
    def NCHUNK(self):
        return 4

    @property
    def QW(self):             # window-quarter boundaries
        base = self.NW // self.NCHUNK
        rem = self.NW - base * self.NCHUNK
        qs = [base + (1 if i < rem else 0) for i in range(self.NCHUNK)]
        return [0] + list(np.cumsum(qs))

    @property
    def QROW(self):           # table row offsets per quarter
        return [self.NCORES * self.P * q for q in self.QW]

    @property
    def NSW(self):
        return (self.NW + self.SW - 1) // self.SW

    @property
    def DIN(self):            # GCN layer-0 input dim
        return self.NFEAT + self.NLABEL

    @property
    def KIN0(self):           # 128-chunks of DIN (padded)
        return (self.DIN + self.P - 1) // self.P


FULL = Cfg()
PAD_DR = 200                 # dst_rel sentinel for padded edge slots

# ----------------------------------------------------------------------------
# host-side preprocessing
# ----------------------------------------------------------------------------


def _build_feats(cfg, x, y, idx_labeled):
    n = x.shape[0]
    idx = np.full((n,), cfg.NLABEL + 2, np.int64)
    idx[idx_labeled] = y[idx_labeled]
    feats = np.zeros((n, cfg.NLABEL), np.float32)
    lab = idx < cfg.NLABEL
    feats[np.nonzero(lab)[0], idx[lab]] = 1.0
    return np.concatenate([x, feats], axis=1)


def _trow(cfg, nodes):
    """Global node id -> table row under the window-quarter layout."""
    QW = np.array(cfg.QW)
    QROW = np.array(cfg.QROW)
    k = nodes // cfg.NOWN
    local = nodes % cfg.NOWN
    w = local // cfg.P
    p = local % cfg.P
    q = np.searchsorted(QW, w, side="right") - 1
    nwq = (QW[q + 1] - QW[q])
    return QROW[q] + k * nwq * cfg.P + (w - QW[q]) * cfg.P + p, q


def _build_schedule(cfg, adj):
    """Device-independent schedule + per-device index/indicator arrays."""
    P = cfg.P
    src = adj[0].astype(np.int64)
    dst = adj[1].astype(np.int64)

    indeg = np.bincount(dst, minlength=cfg.N).astype(np.float32)
    deg_full = indeg + 2.0

    # cells in schedule order: sw asc -> chunk asc -> window asc
    ncells = sum(cfg.NCHUNK * (min((s + 1) * cfg.SW, cfg.NW) - s * cfg.SW)
                 for s in range(cfg.NSW))

    dev_edges = []        # (cell, src_local, dst_rel) arrays per device
    counts = np.zeros((cfg.NCORES, ncells), np.int64)
    sw_sizes = [min((s + 1) * cfg.SW, cfg.NW) - s * cfg.SW
                for s in range(cfg.NSW)]
    sw_off_arr = np.cumsum([0] + [sz * cfg.NCHUNK for sz in sw_sizes])[:-1]
    sw_sz_arr = np.array(sw_sizes)
    QROW = np.array(cfg.QROW)
    for d in range(cfg.NCORES):
        mask = (dst // cfg.NOWN) == d
        es = src[mask]
        ed = dst[mask]
        dl = ed - d * cfg.NOWN
        w = dl // P
        dst_rel = (dl % P).astype(np.uint8)
        trow, c = _trow(cfg, es)
        src_local = (trow - QROW[c]).astype(np.int64)
        sw_of = w // cfg.SW
        w_in_sw = w - sw_of * cfg.SW
        cidv = sw_off_arr[sw_of] + c * sw_sz_arr[sw_of] + w_in_sw
        np.add.at(counts[d], cidv, 1)
        dev_edges.append((cidv, src_local, dst_rel))

    G = (counts.max(axis=0) + P - 1) // P   # groups per cell (0 if empty)
    cap = G * P
    cell_off = np.concatenate([[0], np.cumsum(cap)])       # edge offsets
    G_off = np.concatenate([[0], np.cumsum(G)])            # group offsets
    G_total = int(G.sum())
    total = int(cap.sum())

    # ---- per-sw gather calls + pair stream -------------------------------
    sw_calls = []         # per sw: [(chunk, ng, goff_rel)] (calls split in 2)
    sw_g_off = []         # sw group base (global)
    sw_g_cnt = []
    sw_pairs = []         # per sw: [(j_rel, w, start, stop)]
    sw_pair_off = []      # pair base (global, into indicator tensor)
    pair_groups = []      # global: [(ga, gb)] absolute group id or -1
    ci = 0
    for s in range(cfg.NSW):
        ws = list(range(s * cfg.SW, min((s + 1) * cfg.SW, cfg.NW)))
        g0 = G_off[ci]
        calls = []
        ci_start = ci
        for c in range(cfg.NCHUNK):
            ng = 0
            goff = G_off[ci]
            for _ in ws:
                ng += int(G[ci])
                ci += 1
            goff = int(goff - g0)
            nsplit = 3
            done = 0
            for si in range(nsplit):
                part = (ng - done + nsplit - si - 1) // (nsplit - si)
                if part:
                    calls.append((c, part, goff + done))
                done += part
        gcnt = int(G_off[ci] - g0)
        assert gcnt >= 2, f"superwindow {s} has <2 groups"
        # pair stream in cell order
        stream = []       # (j_rel, w, ga, gb)
        cj = ci_start
        for c in range(cfg.NCHUNK):
            for w in ws:
                off = int(G_off[cj] - g0)
                Gc = int(G[cj])
                for p in range(Gc // 2):
                    stream.append((off + 2 * p, w,
                                   int(G_off[cj]) + 2 * p,
                                   int(G_off[cj]) + 2 * p + 1))
                if Gc % 2:
                    gl = int(G_off[cj]) + Gc - 1
                    if Gc >= 3:
                        stream.append((off + Gc - 2, w, -1, gl))
                    elif off >= 1:
                        stream.append((off - 1, w, -1, gl))
                    else:
                        stream.append((0, w, gl, -1))
                cj += 1
        # start/stop flags per window
        first = {}
        last = {}
        for i, (j, w, ga, gb) in enumerate(stream):
            if w not in first:
                first[w] = i
            last[w] = i
        sw_pairs.append([(j, w, int(i == first[w]), int(i == last[w]))
                         for i, (j, w, ga, gb) in enumerate(stream)])
        sw_pair_off.append(len(pair_groups))
        pair_groups.extend((ga, gb) for (j, w, ga, gb) in stream)
        sw_calls.append(calls)
        sw_g_off.append(int(g0))
        sw_g_cnt.append(gcnt)
    M_total = len(pair_groups)
    PAIRS_MAX = max(len(p) for p in sw_pairs)
    G_SW_MAX = max(sw_g_cnt)
    pair_groups = np.array(pair_groups, np.int64)          # [M, 2]

    # per-device data arrays
    dev_idx = []
    dev_ind = []
    dev_deg = []
    onehot = np.zeros((256, P), F8)
    onehot[np.arange(P), np.arange(P)] = 1.0
    for d in range(cfg.NCORES):
        cidv, src_local, dst_rel = dev_edges[d]
        # ascending src within each cell -> better HBM locality for gathers
        order = np.lexsort((src_local, cidv))
        cid_s = cidv[order]
        starts = np.searchsorted(cid_s, np.arange(ncells))
        within = np.arange(len(cid_s)) - starts[cid_s]
        pos = cell_off[cid_s] + within
        idx_flat = np.zeros(total, np.int64)
        dr_flat = np.full(total, PAD_DR, np.uint8)
        idx_flat[pos] = src_local[order]
        dr_flat[pos] = dst_rel[order]
        # wrapped int16 layout [128, total//16]
        a = idx_flat.reshape(total // 16, 16).T.astype(np.int16)
        dev_idx.append(np.ascontiguousarray(np.tile(a, (8, 1))))
        # indicators: [P(edge), M, 2, P(dst)] fp8
        dr_groups = dr_flat.reshape(G_total, P)
        slot_dr = np.full((M_total, 2, P), PAD_DR, np.uint8)
        for k in range(2):
            gk = pair_groups[:, k]
            valid = gk >= 0
            slot_dr[valid, k, :] = dr_groups[gk[valid]]
        ind = onehot[slot_dr]                   # [M, 2, Pedge, Pdst]
        dev_ind.append(np.ascontiguousarray(ind.transpose(2, 0, 1, 3)))
        dg = np.full((cfg.NOWN_PAD,), 1.0, np.float32)
        dg[:cfg.NOWN] = deg_full[d * cfg.NOWN:(d + 1) * cfg.NOWN]
        dev_deg.append(np.ascontiguousarray(
            dg.reshape(cfg.NW, P).T))          # [128, NW]

    sched = dict(
        sw_calls=sw_calls, sw_g_off=sw_g_off, sw_g_cnt=sw_g_cnt,
        sw_pairs=sw_pairs, sw_pair_off=sw_pair_off,
        G_SW_MAX=int(G_SW_MAX), PAIRS_MAX=int(PAIRS_MAX),
        G_total=G_total, M_total=M_total, S_total=G_total * 8,
    )
    return sched, dev_idx, dev_ind, dev_deg


def _pack_h0(cfg, h0_dev):
    """[NOWN_PAD, DIN] f32 -> [NW, 128, KIN0*128] fp8 lhsT-packed."""
    dpad = cfg.KIN0 * cfg.P
    h = np.zeros((cfg.NOWN_PAD, dpad), np.float32)
    h[:, :cfg.DIN] = h0_dev
    v = h.reshape(cfg.NW, cfg.P, cfg.KIN0, cfg.P)      # t, nc, kc, p
    return np.ascontiguousarray(v.transpose(0, 3, 2, 1)
                                .reshape(cfg.NW, cfg.P, cfg.KIN0 * cfg.P)
                                .astype(F8))


def _pack_w(W, kin_chunks, p=128):
    """[K, O] -> [kin_chunks, 128, O] bf16 (zero-padded)."""
    K, O = W.shape
    Wp = np.zeros((kin_chunks * p, O), np.float32)
    Wp[:K] = W
    return np.ascontiguousarray(
        Wp.reshape(kin_chunks, p, O).astype(BF16))


def _bcast(b, p=128):
    return np.ascontiguousarray(np.broadcast_to(
        b.astype(np.float32)[None, :], (p, len(b))).copy())


# ----------------------------------------------------------------------------
# Bass program
# ----------------------------------------------------------------------------


def _build_nc(cfg, sched):
    from concourse import bass, mybir, tile, bacc
    from concourse.masks import make_identity
    from contextlib import ExitStack

    fp32 = mybir.dt.float32
    bf16 = mybir.dt.bfloat16
    fp8 = mybir.dt.float8e4
    i16 = mybir.dt.int16
    DRmode = mybir.MatmulPerfMode.DoubleRow
    P = cfg.P
    NH = cfg.NHID
    NW = cfg.NW
    SW = cfg.SW
    KIN0 = cfg.KIN0
    QW = cfg.QW
    QROW = cfg.QROW
    G_SW_MAX = sched["G_SW_MAX"]
    PAIRS_MAX = sched["PAIRS_MAX"]
    # AllGather quarter q is issued after this phase-M window index
    ag_after_w = {QW[q + 1] - 1: q for q in range(cfg.NCHUNK)}

    nc = bacc.Bacc("TRN2", debug=False, num_swdge_queues=4)

    hT0_d = nc.dram_tensor("hT0", [NW, P, KIN0 * P], fp8, kind="ExternalInput")
    idx_d = nc.dram_tensor("idx", [P, sched["S_total"]], i16, kind="ExternalInput")
    ind_d = nc.dram_tensor("ind", [P, sched["M_total"], 2, P], fp8,
                           kind="ExternalInput")
    deg_d = nc.dram_tensor("deg", [P, NW], fp32, kind="ExternalInput")
    w0_d = nc.dram_tensor("w0", [KIN0, P, NH], fp8, kind="ExternalInput")
    w12_d = nc.dram_tensor("w12", [2, 2, P, NH], bf16, kind="ExternalInput")
    wm0_d = nc.dram_tensor("wm0", [2, P, 2 * NH], bf16, kind="ExternalInput")
    wm1_d = nc.dram_tensor("wm1", [4, P, 64], bf16, kind="ExternalInput")
    b012_d = nc.dram_tensor("b012", [3, P, NH], fp32, kind="ExternalInput")
    bm0_d = nc.dram_tensor("bm0", [P, 2 * NH], fp32, kind="ExternalInput")
    bm1_d = nc.dram_tensor("bm1", [P, 64], fp32, kind="ExternalInput")
    out_d = nc.dram_tensor("out", [NW, P, 64], fp32, kind="ExternalOutput")

    with tile.TileContext(nc) as tc, ExitStack() as ctx:
        const = ctx.enter_context(tc.tile_pool(name="const", bufs=1))
        ht = ctx.enter_context(tc.tile_pool(name="ht", bufs=1))
        work = ctx.enter_context(tc.tile_pool(name="work", bufs=2))
        tri = ctx.enter_context(tc.tile_pool(name="tri", bufs=2))
        pacc = ctx.enter_context(tc.tile_pool(name="pacc", bufs=6, space="PSUM"))
        pmz = ctx.enter_context(tc.tile_pool(name="pmz", bufs=2, space="PSUM"))
        dram = ctx.enter_context(tc.tile_pool(name="dram", bufs=1, space="DRAM"))

        # ---- constants -----------------------------------------------------
        ident = const.tile([P, P], bf16, tag="ident")
        make_identity(nc, ident[:])
        deg_sb = const.tile([P, NW], fp32, tag="deg")
        nc.sync.dma_start(deg_sb[:], deg_d[:])
        dinv = const.tile([P, NW], fp32, tag="dinv")
        nc.scalar.sqrt(deg_sb[:], deg_sb[:])
        nc.vector.reciprocal(dinv[:], deg_sb[:])
        dinv2 = const.tile([P, NW], fp32, tag="dinv2")
        nc.vector.tensor_scalar_mul(dinv2[:], dinv[:], 2.0)

        w0_sb = const.tile([P, KIN0, NH], fp8, tag="w0")
        nc.sync.dma_start(w0_sb[:], w0_d[:].rearrange("k p o -> p k o"))
        w12_sb = const.tile([P, 2, 2, NH], bf16, tag="w12")
        nc.sync.dma_start(w12_sb[:], w12_d[:].rearrange("l k p o -> p l k o"))
        wm0_sb = const.tile([P, 2, 2 * NH], bf16, tag="wm0")
        nc.sync.dma_start(wm0_sb[:], wm0_d[:].rearrange("k p o -> p k o"))
        wm1_sb = const.tile([P, 4, 64], bf16, tag="wm1")
        nc.sync.dma_start(wm1_sb[:], wm1_d[:].rearrange("k p o -> p k o"))
        b012_sb = const.tile([P, 3, NH], fp32, tag="b012")
        nc.sync.dma_start(b012_sb[:], b012_d[:].rearrange("l p o -> p l o"))
        bm0_sb = const.tile([P, 2 * NH], fp32, tag="bm0")
        nc.sync.dma_start(bm0_sb[:], bm0_d[:])
        bm1_sb = const.tile([P, 64], fp32, tag="bm1")
        nc.sync.dma_start(bm1_sb[:], bm1_d[:])

        # persistent transposed activations, 2 feature chunks of 128
        hTa = ht.tile([P, NW * P], bf16, tag="hTa")
        hTb = ht.tile([P, NW * P], bf16, tag="hTb")

        ag_ins = [dram.tile([NW, P, NH], fp8, tag=f"agin{l}",
                            name=f"agin{l}") for l in range(3)]
        tables = [[dram.tile([QROW[q + 1] - QROW[q], NH], fp8,
                             tag=f"tbl{l}q{q}", name=f"tbl{l}q{q}",
                             addr_space="Shared")
                   for q in range(cfg.NCHUNK)] for l in range(3)]

        def issue_ag(layer, q):
            nc.gpsimd.collective_compute(
                "AllGather", mybir.AluOpType.bypass,
                ins=[ag_ins[layer][QW[q]:QW[q + 1]].opt()],
                outs=[tables[layer][q][:].opt()],
                replica_groups=[list(range(cfg.NCORES))],
            )

        def phase_m_window(layer, t, stage, si):
            """z = h@W for window t -> stage[:, si, :] (fp8, dinv-scaled)."""
            nkin = 2
            psum_z = pacc.tile([P, NH], fp32, tag="acc", name="psum_z")
            for kc in range(nkin):
                lhsT = (hTa if kc == 0 else hTb)[:, t * P:(t + 1) * P]
                rhs = w12_sb[:, layer - 1, kc, :]
                nc.tensor.matmul(psum_z[:], lhsT, rhs,
                                 start=(kc == 0), stop=(kc == nkin - 1))
            nc.vector.tensor_scalar_mul(
                stage[:, si, :], psum_z[:], dinv[:, t:t + 1])

        def head_windows(w_lo, w_hi):
            """MLP head + log_softmax for windows [w_lo, w_hi)."""
            nwv = w_hi - w_lo
            mbs = []
            for j in range(nwv):
                t = w_lo + j
                psum_m = pacc.tile([P, 2 * NH], fp32, tag="acc",
                                   name="psum_m")
                for kc in range(2):
                    lhsT = (hTa if kc == 0 else hTb)[:, t * P:(t + 1) * P]
                    nc.tensor.matmul(psum_m[:], lhsT, wm0_sb[:, kc, :],
                                     start=(kc == 0), stop=(kc == 1))
                z0 = tri.tile([P, 2 * NH], fp32, tag="z0")
                nc.vector.tensor_add(z0[:], psum_m[:], bm0_sb[:])
                # elu(z) = relu(z) + min(exp(z) - 1, 0)
                ex = tri.tile([P, 2 * NH], bf16, tag="ex")
                nc.scalar.activation(ex[:], z0[:],
                                     mybir.ActivationFunctionType.Exp)
                nc.vector.tensor_scalar(
                    out=ex[:], in0=ex[:], scalar1=1.0, scalar2=0.0,
                    op0=mybir.AluOpType.subtract, op1=mybir.AluOpType.min)
                mb = tri.tile([P, 2 * NH], bf16, tag="mb", name="mb",
                              bufs=SW + 2)
                nc.scalar.activation(mb[:], z0[:],
                                     mybir.ActivationFunctionType.Relu)
                nc.vector.tensor_add(mb[:], mb[:], ex[:])
                mbs.append(mb)
            lg8 = tri.tile([P, SW, 64], fp32, tag="lg8")
            for j in range(nwv):
                mT = tri.tile([P, 4, P], bf16, tag="mT", name="mT")
                for q in range(4):
                    ptp = pmz.tile([P, P], bf16, tag="mz", name="ptp")
                    nc.tensor.transpose(ptp[:], mbs[j][:, q * P:(q + 1) * P],
                                        ident[:])
                    nc.scalar.activation(mT[:, q, :], ptp[:],
                                         mybir.ActivationFunctionType.Copy)
                psum_l = pacc.tile([P, 64], fp32, tag="acc", name="psum_l")
                for q in range(4):
                    nc.tensor.matmul(psum_l[:], mT[:, q, :], wm1_sb[:, q, :],
                                     start=(q == 0), stop=(q == 3))
                nc.vector.tensor_add(lg8[:, j, :], psum_l[:], bm1_sb[:])
            # batched log_softmax over the wave
            mx8 = tri.tile([P, SW, 1], fp32, tag="mx8")
            nc.vector.tensor_reduce(mx8[:, :nwv, :], lg8[:, :nwv, :],
                                    axis=mybir.AxisListType.X,
                                    op=mybir.AluOpType.max)
            nc.vector.tensor_tensor(
                out=lg8[:, :nwv, :], in0=lg8[:, :nwv, :],
                in1=mx8[:, :nwv, :].to_broadcast([P, nwv, 64]),
                op=mybir.AluOpType.subtract)
            ex8 = tri.tile([P, SW, 64], bf16, tag="ex8")
            nc.scalar.activation(ex8[:, :nwv, :], lg8[:, :nwv, :],
                                 mybir.ActivationFunctionType.Exp)
            se8 = tri.tile([P, SW, 1], fp32, tag="se8")
            nc.vector.tensor_reduce(se8[:, :nwv, :], ex8[:, :nwv, :],
                                    axis=mybir.AxisListType.X,
                                    op=mybir.AluOpType.add)
            ln8 = tri.tile([P, SW, 1], fp32, tag="ln8")
            nc.scalar.activation(ln8[:, :nwv, :], se8[:, :nwv, :],
                                 mybir.ActivationFunctionType.Ln)
            out_stage = tri.tile([P, SW, 64], fp32, tag="ostage")
            nc.vector.tensor_tensor(
                out=out_stage[:, :nwv, :], in0=lg8[:, :nwv, :],
                in1=ln8[:, :nwv, :].to_broadcast([P, nwv, 64]),
                op=mybir.AluOpType.subtract)
            nc.sync.dma_start(
                out_d[w_lo:w_hi].rearrange("t p f -> p t f"),
                out_stage[:, :nwv, :])

        # ---- layer-0 phase M (standalone, AG quarters issued inline) ------
        SLAB = 4
        g_stage = None
        stage_base = 0
        h0slab = None
        for t in range(NW):
            if t % SLAB == 0:
                nsl = min(SLAB, NW - t)
                h0slab = tri.tile([P, SLAB, KIN0, P], fp8,
                                  tag="h0slab", bufs=2)
                nc.sync.dma_start(
                    h0slab[:, :nsl, :, :],
                    hT0_d[t:t + nsl].rearrange("t p (k q) -> p t k q", q=P))
            if t == stage_base:
                g_stage = tri.tile([P, 8, NH], fp8, tag="stage")
            psum_z = pacc.tile([P, NH], fp32, tag="acc", name="psum_z")
            for kp in range(KIN0 // 2):
                nc.tensor.matmul(psum_z[:],
                                 h0slab[:, t % SLAB, 2 * kp:2 * kp + 2, :],
                                 w0_sb[:, 2 * kp:2 * kp + 2, :],
                                 start=(kp == 0), stop=False,
                                 perf_mode=DRmode)
            nc.tensor.matmul(psum_z[:],
                             h0slab[:, t % SLAB, KIN0 - 1, :],
                             w0_sb[:, KIN0 - 1, :],
                             start=False, stop=True)
            nc.vector.tensor_scalar_mul(
                g_stage[:, t - stage_base, :], psum_z[:], dinv[:, t:t + 1])
            flush = (t - stage_base == 7) or (t == NW - 1) or (t in ag_after_w)
            if flush:
                nb = t - stage_base + 1
                nc.sync.dma_start(
                    ag_ins[0][stage_base:stage_base + nb]
                    .rearrange("t p f -> p t f"),
                    g_stage[:, :nb, :])
                stage_base = t + 1
            if t in ag_after_w:
                issue_ag(0, ag_after_w[t])

        # ---- 3 GCN layers: phase A (+ next phase M / head interleaved) ----
        for layer in range(3):
            table = tables[layer]
            ag_in = ag_ins[layer]

            def postlude(w, acc, gown, w_lo):
                # h = relu(acc*dinv + gown*2*dinv + bias); -> hTa/hTb
                tmp = tri.tile([P, NH], fp32, tag="pl_tmp", name="pl_tmp")
                if acc is not None:
                    nc.vector.scalar_tensor_tensor(
                        out=tmp[:], in0=acc[:],
                        scalar=dinv[:, w:w + 1],
                        in1=b012_sb[:, layer, :],
                        op0=mybir.AluOpType.mult,
                        op1=mybir.AluOpType.add)
                    nc.vector.scalar_tensor_tensor(
                        out=tmp[:], in0=gown[:, w - w_lo, :],
                        scalar=dinv2[:, w:w + 1],
                        in1=tmp[:],
                        op0=mybir.AluOpType.mult,
                        op1=mybir.AluOpType.add)
                else:
                    nc.vector.scalar_tensor_tensor(
                        out=tmp[:], in0=gown[:, w - w_lo, :],
                        scalar=dinv2[:, w:w + 1],
                        in1=b012_sb[:, layer, :],
                        op0=mybir.AluOpType.mult,
                        op1=mybir.AluOpType.add)
                hbf = tri.tile([P, NH], bf16, tag="pl_hbf", name="pl_hbf")
                nc.scalar.activation(
                    hbf[:], tmp[:], mybir.ActivationFunctionType.Relu)
                for half, dst_t in ((0, hTa), (1, hTb)):
                    ptp = pmz.tile([P, P], bf16, tag="mz", name="ptp")
                    nc.tensor.transpose(
                        ptp[:], hbf[:, half * P:(half + 1) * P], ident[:])
                    nc.scalar.activation(
                        dst_t[:, w * P:(w + 1) * P], ptp[:],
                        mybir.ActivationFunctionType.Copy)

            for s in range(cfg.NSW):
                w_lo = s * SW
                w_hi = min((s + 1) * SW, NW)
                nwin = w_hi - w_lo
                g0 = sched["sw_g_off"][s]
                gcnt = sched["sw_g_cnt"][s]
                pairs = sched["sw_pairs"][s]
                p0 = sched["sw_pair_off"][s]

                idx_sb = work.tile([P, G_SW_MAX * 8], i16, tag="idx")
                nc.sync.dma_start(idx_sb[:, :gcnt * 8],
                                  idx_d[:, g0 * 8:(g0 + gcnt) * 8])
                ind_sb = work.tile([P, PAIRS_MAX, 2, P], fp8, tag="ind")
                nc.sync.dma_start(ind_sb[:, :len(pairs), :, :],
                                  ind_d[:, p0:p0 + len(pairs), :, :])
                gown = work.tile([P, SW, NH], fp8, tag="gown", bufs=3)
                nc.sync.dma_start(
                    gown[:, :nwin, :],
                    ag_in[w_lo:w_hi].rearrange("t p f -> p t f"))
                gath = work.tile([P, G_SW_MAX + 1, NH], fp8, tag="gath",
                                 bufs=5)
                for ci_call, (c, ng, goff) in enumerate(
                        sched["sw_calls"][s]):
                    nc.gpsimd.dma_gather(
                        out_ap=gath[:, goff:goff + ng, :],
                        in_ap=table[c][:],
                        idxs_ap=idx_sb[:, goff * 8:(goff + ng) * 8],
                        num_idxs=ng * P,
                        num_idxs_reg=ng * P,
                        elem_size=NH,
                        single_packet=True,
                        queue_num=ci_call % 4,
                    )
                accs = {}
                for pl, (j, w, st, sp) in enumerate(pairs):
                    if st:
                        accs[w] = pacc.tile([P, NH], fp32, tag="acc",
                                            name="acc")
                    nc.tensor.matmul(
                        accs[w][:], ind_sb[:, pl, :, :],
                        gath[:, j:j + 2, :],
                        start=bool(st), stop=bool(sp), perf_mode=DRmode)
                    if sp:
                        postlude(w, accs[w], gown, w_lo)
                for w in range(w_lo, w_hi):
                    if w not in accs:
                        postlude(w, None, gown, w_lo)

                # interleaved next-layer phase M / final head
                if layer < 2:
                    stage = tri.tile([P, SW, NH], fp8, tag="stage2")
                    for w in range(w_lo, w_hi):
                        phase_m_window(layer + 1, w, stage, w - w_lo)
                    nc.sync.dma_start(
                        ag_ins[layer + 1][w_lo:w_hi]
                        .rearrange("t p f -> p t f"),
                        stage[:, :nwin, :])
                    for w in range(w_lo, w_hi):
                        if w in ag_after_w:
                            issue_ag(layer + 1, ag_after_w[w])
                else:
                    head_windows(w_lo, w_hi)

    nc.compile()
    return nc


# ----------------------------------------------------------------------------
# entry point
# ----------------------------------------------------------------------------

_NC_CACHE = {}
TRACE = False
TRACE_KW = {}
LAST_RESULT = None


def _prepare(cfg, inputs):
    x = np.asarray(inputs["x"], np.float32)
    y = np.asarray(inputs["y"])
    adj = np.asarray(inputs["adj"])
    idx_labeled = np.asarray(inputs["idx_labeled"])

    h0 = _build_feats(cfg, x, y, idx_labeled)
    sched, dev_idx, dev_ind, dev_deg = _build_schedule(cfg, adj)

    W0 = _pack_w(np.asarray(inputs["W0"], np.float32),
                 cfg.KIN0).astype(F8)
    W1 = _pack_w(np.asarray(inputs["W1"], np.float32), 2)
    W2 = _pack_w(np.asarray(inputs["W2"], np.float32), 2)
    w12 = np.ascontiguousarray(np.stack([W1, W2]))
    Wm0 = _pack_w(np.asarray(inputs["Wm0"], np.float32), 2)
    Wm1 = _pack_w(np.asarray(inputs["Wm1"], np.float32), 4)
    b012 = np.ascontiguousarray(np.stack(
        [_bcast(np.asarray(inputs[k], np.float32)) for k in ("b0", "b1", "b2")]))
    bm0 = _bcast(np.asarray(inputs["bm0"], np.float32))
    bm1 = _bcast(np.asarray(inputs["bm1"], np.float32))

    in_maps = []
    for d in range(cfg.NCORES):
        h0_dev = np.zeros((cfg.NOWN_PAD, cfg.DIN), np.float32)
        h0_dev[:cfg.NOWN] = h0[d * cfg.NOWN:(d + 1) * cfg.NOWN]
        in_maps.append(dict(
            hT0=_pack_h0(cfg, h0_dev),
            idx=dev_idx[d], ind=dev_ind[d], deg=dev_deg[d],
            w0=W0, w12=w12, wm0=Wm0, wm1=Wm1,
            b012=b012, bm0=bm0, bm1=bm1,
        ))
    return sched, in_maps


def run(cfg, inputs):
    global LAST_RESULT
    from concourse.bass_utils import run_bass_kernel_spmd

    sched, in_maps = _prepare(cfg, inputs)
    key = (cfg, hashlib.sha1(
        np.asarray(inputs["adj"]).tobytes()).hexdigest())
    if key not in _NC_CACHE:
        _NC_CACHE[key] = _build_nc(cfg, sched)
    nc = _NC_CACHE[key]

    res = run_bass_kernel_spmd(
        nc, in_maps, core_ids=list(range(cfg.NCORES)),
        trace=TRACE, **TRACE_KW)
    LAST_RESULT = res
    outs = []
    for d in range(cfg.NCORES):
        o = res.results[d]["out"].reshape(cfg.NOWN_PAD, 64)
        outs.append(o[:cfg.NOWN])
    return np.ascontiguousarray(np.concatenate(outs, axis=0))


def kernel(**inputs) -> np.ndarray:
    return run(FULL, inputs)
